# revision 10
# baseline (speedup 1.0000x reference)
"""Trainium2 Bass kernel for autoregressive Bernoulli sampling (AP_NN).

Strategy:
- Batch rows (64) sharded 8 per NeuronCore (pure data parallelism).
- The jax PRNG is data-independent: host precomputes, for every (t, b),
  the exact f32 threshold theta s.t.  (u < sigmoid_jax(x)) <=> (x >= theta).
  Device keeps the margin state y = x - theta and decides spikes by sign.
- Device scan per core: for each step j, one compare (spike = y_j >= 0)
  and one fused (ktail * spike + y) scatter over the next 127 columns
  (lags 1..127) on VectorE; lags 128..501 are folded in per 128-step chunk
  with TensorE matmuls over the transposed spike history.
- Outputs: S = spikes (exact), P = sigmoid(y + theta) via ScalarE.
"""

import os
import numpy as np

T_NO = 501
COS_BASIS_NO = 30
SCALE = 7.5
SHIFT = 1
B, T = 64, 2048
NCORES = 8
RPC = B // NCORES  # 8 rows per core
C = 128            # chunk length
NCHUNK = T // C    # 16
W = C - 1          # per-step scatter window (lags 1..127)

_CACHE = {}
LAST_RESULT = None  # BassKernelResults of the most recent device run


# ---------------------------------------------------------------- host tables
def _f2ord(x):
    ub = x.view(np.uint32).astype(np.int64)
    return np.where(ub >= 0x80000000, np.int64(0xFFFFFFFF) - ub, ub + np.int64(0x80000000))


def _ord2f(o):
    o = o.astype(np.int64)
    ub = np.where(o >= np.int64(0x80000000), o - np.int64(0x80000000), np.int64(0xFFFFFFFF) - o)
    return ub.astype(np.uint32).view(np.float32)


def _theta():
    """[T, B] f32: exact spike thresholds from the jax PRNG stream."""
    if "theta" in _CACHE:
        return _CACHE["theta"]
    import jax
    import jax.numpy as jnp

    cpu = jax.devices("cpu")[0]
    with jax.default_device(cpu):
        def uchain(key, _):
            key, sub = jax.random.split(key)
            return key, jax.random.uniform(sub, (B,), jnp.float32)

        _, u = jax.lax.scan(uchain, jax.random.key(42), None, length=T)
        u = np.asarray(u)  # [T, B]
        sigf = jax.jit(jax.nn.sigmoid, backend="cpu")

        def sig_np(x):
            return np.asarray(sigf(jnp.asarray(x, jnp.float32)))

    lo = np.full(u.shape, np.float32(-40.0))
    hi = np.full(u.shape, np.float32(40.0))
    lo_o, hi_o = _f2ord(lo), _f2ord(hi)
    while True:
        if (hi_o - lo_o <= 1).all():
            break
        mid_o = (lo_o + hi_o) // 2
        gt = sig_np(_ord2f(mid_o)) > u
        lo_o = np.where(gt, lo_o, mid_o)
        hi_o = np.where(gt, mid_o, hi_o)
    theta = _ord2f(hi_o)  # smallest f32 with sigmoid(theta) > u
    _CACHE["theta"] = (theta, u)
    return _CACHE["theta"]


def _refract_kern(W_refract):
    i = np.arange(COS_BASIS_NO, dtype=np.float64)[:, None]
    phi = 0.5 * np.pi * i
    x = np.arange(T_NO, dtype=np.float64)[None, :]
    raw = SCALE * np.log(x + SHIFT + 1e-7)
    basis = 0.5 * np.cos(raw - phi) + 0.5
    basis = np.where((raw < phi - np.pi) | (raw > phi + np.pi), 0.0, basis).astype(np.float32)
    rk = (basis.T @ W_refract.astype(np.float32)).astype(np.float32)[::-1]
    return np.ascontiguousarray(rk)  # [T_NO], flipped as in reference


def _nn_host(V, D, w1, b1, w2, b2):
    """Pointwise MLP in f64 (<< margin below the f32 reference)."""
    a = w1[:, 0, 0].astype(np.float64)
    bb = w1[:, 1, 0].astype(np.float64)
    h = np.tanh(a[:, None, None] * V.astype(np.float64) + bb[:, None, None] * D.astype(np.float64)
                + b1.astype(np.float64)[:, None, None])
    nn = (w2[0, :, 0].astype(np.float64)[:, None, None] * h).sum(0) + np.float64(b2[0])
    return nn  # [B, T] f64


# ---------------------------------------------------------------- bass kernel
def _spike_op():
    """Register (once) the fused scatter op: out = in0 + in1 * (s0 >= 0).

    One DVE instruction per scan step: the spike decision (sign of the
    per-partition scalar s0 = y[:, j]) and the weighted scatter into the
    next W columns, with no cross-instruction scalar hazard."""
    if "op" in _CACHE:
        return _CACHE["op"]
    import concourse.dve_ops as dops
    from concourse.dve_spec import Spec, Src0, Src1, C0, Zero, lower
    from concourse.dve_uop import DveOpSpec

    name = "SPIKE_SCATTER_ANT"
    for o in dops.OPS:
        if o.name == name:
            _CACHE["op"] = o
            return o
    # (C0 >= Src0*Zero) == (C0 >= 0) elementwise, but keeps the compare in
    # the streaming stages (IS_GE has no swap-flop complement for latch-init).
    spec = Spec(
        body=Src0 + Src1 * (C0 >= Src0 * Zero),
        reference=lambda in0, in1, s0, s1, imm2: (
            in0 + in1 * (s0 >= 0).astype(np.float32)).astype(np.float32),
    )
    opcode = dops._CUSTOM_DVE_ROW_BASE + len(dops.OPS)
    shas = {}
    for ver in ("v3", "v4"):
        shas[ver] = DveOpSpec(name=name, opcode=opcode,
                              uops=lower(spec, ver=ver), rd1_en=True).sha(ver)
    op = dops.DveOp(name, spec, subdim=False, uops_sha=shas)
    dops.OPS.append(op)
    dops._SUB_OPCODE_FOR_NAME[name] = opcode
    dops.CUSTOM_DVE_SPECS[name] = spec
    _CACHE["op"] = op
    return op


def _build_nc():
    if "nc" in _CACHE:
        return _CACHE["nc"]
    import concourse.bass as bass
    import concourse.mybir as mybir
    from contextlib import ExitStack

    f32 = mybir.dt.float32
    nc = bass.Bass()

    y_d = nc.dram_tensor("y0", [RPC, T], f32, kind="ExternalInput")
    th_d = nc.dram_tensor("theta", [RPC, T], f32, kind="ExternalInput")
    kt_d = nc.dram_tensor("ktail", [RPC, T_NO + 1], f32, kind="ExternalInput")
    k4_d = nc.dram_tensor("k4", [C, 4 * C], f32, kind="ExternalInput")
    id_d = nc.dram_tensor("ident", [RPC, RPC], f32, kind="ExternalInput")
    out_d = nc.dram_tensor("out", [2, RPC, T], f32, kind="ExternalOutput")

    ctx = ExitStack()
    y = ctx.enter_context(nc.sbuf_tensor([RPC, T + C], f32))  # +C scratch pad: full-width scatters
    th = ctx.enter_context(nc.sbuf_tensor([RPC, T], f32))
    S = ctx.enter_context(nc.sbuf_tensor([RPC, T], f32))
    kt = ctx.enter_context(nc.sbuf_tensor([RPC, T_NO + 1], f32))
    k4 = ctx.enter_context(nc.sbuf_tensor([C, 4 * C], f32))
    ident = ctx.enter_context(nc.sbuf_tensor([RPC, RPC], f32))
    spT = ctx.enter_context(nc.sbuf_tensor([C, NCHUNK * RPC], f32))
    zb = ctx.enter_context(nc.sbuf_tensor([RPC, 1], f32))
    dps = ctx.enter_context(nc.psum_tensor([RPC, C], f32))
    tps = ctx.enter_context(nc.psum_tensor([C, RPC], f32))

    dma = ctx.enter_context(nc.semaphore())
    s_cmp = ctx.enter_context(nc.semaphore())   # V: chunk m spikes in S       (count m+1)
    s_tr = ctx.enter_context(nc.semaphore())    # T: transpose of chunk m-1    (count m)
    s_spt = ctx.enter_context(nc.semaphore())   # A: spT slice for chunk m-1   (count m)
    s_mm = ctx.enter_context(nc.semaphore())    # T: delta psum for chunk m    (count m)
    s_x = ctx.enter_context(nc.semaphore())     # V: x = y + theta done
    s_p = ctx.enter_context(nc.semaphore())     # A: P sigmoid done

    ge = mybir.AluOpType.is_ge
    mult = mybir.AluOpType.mult
    add = mybir.AluOpType.add

    with nc.Block() as block:

        @block.sync
        def _(sync):
            sync.dma_start(out=y[:, 0:T], in_=y_d[:, :]).then_inc(dma, 16)
            sync.dma_start(out=th[:, :], in_=th_d[:, :]).then_inc(dma, 16)
            sync.dma_start(out=kt[:, :], in_=kt_d[:, :]).then_inc(dma, 16)
            sync.dma_start(out=k4[:, :], in_=k4_d[:, :]).then_inc(dma, 16)
            sync.dma_start(out=ident[:, :], in_=id_d[:, :]).then_inc(dma, 16)
            sync.wait_ge(s_cmp, NCHUNK)
            sync.dma_start(out=out_d[0, :, :], in_=S[:, :]).then_inc(dma, 16)
            sync.wait_ge(s_p, 1)
            sync.dma_start(out=out_d[1, :, :], in_=th[:, :]).then_inc(dma, 16)

        @block.vector
        def _(vector):
            vector.wait_ge(dma, 80)  # ALL input DMAs (completion order varies)
            vector.memset(zb[:, :], 0.0)
            for m in range(NCHUNK):
                t0 = m * C
                if m >= 1:
                    vector.wait_ge(s_mm, m)
                    vector.tensor_add(y[:, t0:t0 + C], y[:, t0:t0 + C], dps[:, :])
                last = None
                for j in range(t0, t0 + C):
                    last = vector.tensor_scalar(S[:, j:j + 1], y[:, j:j + 1], 0.0, None, ge)
                    # drain: the scatter reads S[:, j] as its per-partition
                    # scalar at cycle 0; without a pipe flush that read
                    # races the cmp's SBUF write-ack (~58 cyc).
                    vector.drain()
                    # full-width scatter into the +C pad keeps every op's
                    # timing uniform (clipped tails would let the next cmp
                    # race this op's first-column write-ack).
                    last = vector.scalar_tensor_tensor(
                        y[:, j + 1:j + 1 + W], kt[:, 1:1 + W], S[:, j:j + 1],
                        y[:, j + 1:j + 1 + W], mult, add)
                last.then_inc(s_cmp)  # chunk m spikes complete
            x_i = vector.tensor_add(th[:, :], y[:, 0:T], th[:, :])
            x_i.then_inc(s_x)

        @block.tensor
        def _(tensor):
            tensor.wait_ge(dma, 80)  # k4 + ident loaded
            for m in range(1, NCHUNK):
                tensor.wait_ge(s_cmp, m)       # chunk m-1 spikes ready
                if m >= 2:
                    tensor.wait_ge(s_spt, m - 1)  # tps WAR: copy of m-2 done
                tensor.transpose(tps[:, :], S[:, (m - 1) * C:m * C], ident[:, :]).then_inc(s_tr)
                tensor.wait_ge(s_spt, m)       # spT slice m-1 written
                qs = [q for q in range(4) if m - 4 + q >= 0]
                for qi, q in enumerate(qs):
                    mm = tensor.matmul(
                        dps[:, :],
                        spT[:, (m - 4 + q) * RPC:(m - 3 + q) * RPC],
                        k4[:, q * C:(q + 1) * C],
                        start=(qi == 0), stop=(qi == len(qs) - 1))
                mm.then_inc(s_mm)

        @block.scalar
        def _(scalar):
            for m in range(1, NCHUNK):
                scalar.wait_ge(s_tr, m)
                scalar.activation(spT[:, (m - 1) * RPC:m * RPC], tps[:, :],
                                  mybir.ActivationFunctionType.Copy).then_inc(s_spt)
            scalar.wait_ge(s_x, 1)
            scalar.activation(th[:, :], th[:, :],
                              mybir.ActivationFunctionType.Sigmoid, bias=zb[:, 0:1]).then_inc(s_p)

    ctx.close()
    _CACHE["nc"] = nc
    return nc


def _install_ntff_hook():
    """Shim antenv.axon_hooks (absent in this image) so BASS_TRACE works."""
    import sys, types
    try:
        import antenv.axon_hooks  # noqa: F401
        return
    except ImportError:
        pass
    try:
        if "/root/.axon_site" not in sys.path:
            sys.path.insert(0, "/root/.axon_site")
        from trn_agent_boot.trn_boot import _ntff_profile_via_ctypes
        hook = _ntff_profile_via_ctypes("/opt/axon/libaxon_pjrt.so")
        import antenv
        mod = types.ModuleType("antenv.axon_hooks")
        mod.get_axon_ntff_profile_hook = lambda: hook
        mod.set_axon_ntff_profile_hook = lambda h: None
        antenv.axon_hooks = mod
        sys.modules["antenv.axon_hooks"] = mod
    except Exception:
        pass


# ---------------------------------------------------------------- entry point
def kernel(V, D, w1, b1, w2, b2, W_refract):
    import sys
    if "/opt/trn_rl_repo" not in sys.path:
        sys.path.insert(0, "/opt/trn_rl_repo")
    from concourse.bass_utils import run_bass_kernel_spmd
    global LAST_RESULT
    _install_ntff_hook()

    V = np.asarray(V, np.float32)
    D = np.asarray(D, np.float32)
    theta, _u = _theta()                       # [T, B]
    nn64 = _nn_host(V, D, np.asarray(w1), np.asarray(b1), np.asarray(w2), np.asarray(b2))
    y0 = (nn64 - theta.T.astype(np.float64)).astype(np.float32)   # [B, T]
    th_bt = np.ascontiguousarray(theta.T.astype(np.float32))      # [B, T]

    rk = _refract_kern(np.asarray(W_refract))  # [501]
    wscat = np.zeros(T_NO + 1, np.float32)
    wscat[1:] = rk[::-1]                       # wscat[d] = rk[501-d]
    ktl = np.tile(wscat[None, :], (RPC, 1))    # [8, 502]

    # K4[p, q*C + i] = wscat[512 + i - q*128 - p] for lag in [128, 501]
    k4 = np.zeros((C, 4 * C), np.float32)
    p = np.arange(C)[:, None]
    i = np.arange(C)[None, :]
    for q in range(4):
        lag = 512 + i - q * 128 - p
        valid = (lag >= C) & (lag <= T_NO)
        k4[:, q * C:(q + 1) * C] = np.where(valid, wscat[np.clip(lag, 0, T_NO)], 0.0)
    ident = np.eye(RPC, dtype=np.float32)

    nc = _build_nc()
    in_maps = []
    for c in range(NCORES):
        r = slice(c * RPC, (c + 1) * RPC)
        in_maps.append({
            "y0": np.ascontiguousarray(y0[r]),
            "theta": np.ascontiguousarray(th_bt[r]),
            "ktail": ktl,
            "k4": k4,
            "ident": ident,
        })
    res = run_bass_kernel_spmd(nc, in_maps, core_ids=list(range(NCORES)))
    LAST_RESULT = res
    S = np.concatenate([res.results[c]["out"][0] for c in range(NCORES)], 0)
    P = np.concatenate([res.results[c]["out"][1] for c in range(NCORES)], 0)
    return S, P


# revision 12
# speedup vs baseline: 1.0989x; 1.0989x over previous
"""Trainium2 Bass kernel for autoregressive Bernoulli sampling (AP_NN).

Strategy:
- Batch rows (64) sharded 8 per NeuronCore (pure data parallelism).
- The jax PRNG is data-independent: host precomputes, for every (t, b),
  the exact f32 threshold theta s.t.  (u < sigmoid_jax(x)) <=> (x >= theta).
  Device keeps the margin state y = x - theta and decides spikes by sign.
- Device scan per core: for each step j, one compare (spike = y_j >= 0)
  and one fused (ktail * spike + y) scatter over the next 127 columns
  (lags 1..127) on VectorE; lags 128..501 are folded in per 128-step chunk
  with TensorE matmuls over the transposed spike history.
- Outputs: S = spikes (exact), P = sigmoid(y + theta) via ScalarE.
"""

import os
import numpy as np

T_NO = 501
COS_BASIS_NO = 30
SCALE = 7.5
SHIFT = 1
B, T = 64, 2048
NCORES = 8
RPC = B // NCORES  # 8 rows per core
C = 128            # chunk length
NCHUNK = T // C    # 16
W = C - 1          # per-step scatter window (lags 1..127)

_CACHE = {}
LAST_RESULT = None  # BassKernelResults of the most recent device run


# ---------------------------------------------------------------- host tables
def _f2ord(x):
    ub = x.view(np.uint32).astype(np.int64)
    return np.where(ub >= 0x80000000, np.int64(0xFFFFFFFF) - ub, ub + np.int64(0x80000000))


def _ord2f(o):
    o = o.astype(np.int64)
    ub = np.where(o >= np.int64(0x80000000), o - np.int64(0x80000000), np.int64(0xFFFFFFFF) - o)
    return ub.astype(np.uint32).view(np.float32)


def _theta():
    """[T, B] f32: exact spike thresholds from the jax PRNG stream."""
    if "theta" in _CACHE:
        return _CACHE["theta"]
    import jax
    import jax.numpy as jnp

    cpu = jax.devices("cpu")[0]
    with jax.default_device(cpu):
        def uchain(key, _):
            key, sub = jax.random.split(key)
            return key, jax.random.uniform(sub, (B,), jnp.float32)

        _, u = jax.lax.scan(uchain, jax.random.key(42), None, length=T)
        u = np.asarray(u)  # [T, B]
        sigf = jax.jit(jax.nn.sigmoid, backend="cpu")

        def sig_np(x):
            return np.asarray(sigf(jnp.asarray(x, jnp.float32)))

    lo = np.full(u.shape, np.float32(-40.0))
    hi = np.full(u.shape, np.float32(40.0))
    lo_o, hi_o = _f2ord(lo), _f2ord(hi)
    while True:
        if (hi_o - lo_o <= 1).all():
            break
        mid_o = (lo_o + hi_o) // 2
        gt = sig_np(_ord2f(mid_o)) > u
        lo_o = np.where(gt, lo_o, mid_o)
        hi_o = np.where(gt, mid_o, hi_o)
    theta = _ord2f(hi_o)  # smallest f32 with sigmoid(theta) > u
    _CACHE["theta"] = (theta, u)
    return _CACHE["theta"]


def _refract_kern(W_refract):
    i = np.arange(COS_BASIS_NO, dtype=np.float64)[:, None]
    phi = 0.5 * np.pi * i
    x = np.arange(T_NO, dtype=np.float64)[None, :]
    raw = SCALE * np.log(x + SHIFT + 1e-7)
    basis = 0.5 * np.cos(raw - phi) + 0.5
    basis = np.where((raw < phi - np.pi) | (raw > phi + np.pi), 0.0, basis).astype(np.float32)
    rk = (basis.T @ W_refract.astype(np.float32)).astype(np.float32)[::-1]
    return np.ascontiguousarray(rk)  # [T_NO], flipped as in reference


def _nn_host(V, D, w1, b1, w2, b2):
    """Pointwise MLP in f64 (<< margin below the f32 reference)."""
    a = w1[:, 0, 0].astype(np.float64)
    bb = w1[:, 1, 0].astype(np.float64)
    h = np.tanh(a[:, None, None] * V.astype(np.float64) + bb[:, None, None] * D.astype(np.float64)
                + b1.astype(np.float64)[:, None, None])
    nn = (w2[0, :, 0].astype(np.float64)[:, None, None] * h).sum(0) + np.float64(b2[0])
    return nn  # [B, T] f64


# ---------------------------------------------------------------- bass kernel
def _spike_op():
    """Register (once) the fused scatter op: out = in0 + in1 * (s0 >= 0).

    One DVE instruction per scan step: the spike decision (sign of the
    per-partition scalar s0 = y[:, j]) and the weighted scatter into the
    next W columns, with no cross-instruction scalar hazard."""
    if "op" in _CACHE:
        return _CACHE["op"]
    import concourse.dve_ops as dops
    from concourse.dve_spec import Spec, Src0, Src1, C0, Zero, lower
    from concourse.dve_uop import DveOpSpec

    name = "SPIKE_SCATTER_ANT"
    for o in dops.OPS:
        if o.name == name:
            _CACHE["op"] = o
            return o
    # (C0 >= Src0*Zero) == (C0 >= 0) elementwise, but keeps the compare in
    # the streaming stages (IS_GE has no swap-flop complement for latch-init).
    spec = Spec(
        body=Src0 + Src1 * (C0 >= Src0 * Zero),
        reference=lambda in0, in1, s0, s1, imm2: (
            in0 + in1 * (s0 >= 0).astype(np.float32)).astype(np.float32),
    )
    opcode = dops._CUSTOM_DVE_ROW_BASE + len(dops.OPS)
    shas = {}
    for ver in ("v3", "v4"):
        shas[ver] = DveOpSpec(name=name, opcode=opcode,
                              uops=lower(spec, ver=ver), rd1_en=True).sha(ver)
    op = dops.DveOp(name, spec, subdim=False, uops_sha=shas)
    dops.OPS.append(op)
    dops._SUB_OPCODE_FOR_NAME[name] = opcode
    dops.CUSTOM_DVE_SPECS[name] = spec
    _CACHE["op"] = op
    return op


def _build_nc():
    if "nc" in _CACHE:
        return _CACHE["nc"]
    import concourse.bass as bass
    import concourse.mybir as mybir
    from contextlib import ExitStack

    f32 = mybir.dt.float32
    nc = bass.Bass()

    y_d = nc.dram_tensor("y0", [RPC, T], f32, kind="ExternalInput")
    th_d = nc.dram_tensor("theta", [RPC, T], f32, kind="ExternalInput")
    kt_d = nc.dram_tensor("ktail", [RPC, T_NO + 1], f32, kind="ExternalInput")
    k4_d = nc.dram_tensor("k4", [C, 4 * C], f32, kind="ExternalInput")
    id_d = nc.dram_tensor("ident", [RPC, RPC], f32, kind="ExternalInput")
    out_d = nc.dram_tensor("out", [2, RPC, T], f32, kind="ExternalOutput")

    ctx = ExitStack()
    y = ctx.enter_context(nc.sbuf_tensor([RPC, T + C], f32))  # +C scratch pad: full-width scatters
    th = ctx.enter_context(nc.sbuf_tensor([RPC, T], f32))
    S = ctx.enter_context(nc.sbuf_tensor([RPC, T], f32))
    kt = ctx.enter_context(nc.sbuf_tensor([RPC, T_NO + 1], f32))
    k4 = ctx.enter_context(nc.sbuf_tensor([C, 4 * C], f32))
    ident = ctx.enter_context(nc.sbuf_tensor([RPC, RPC], f32))
    spT = ctx.enter_context(nc.sbuf_tensor([C, NCHUNK * RPC], f32))
    zb = ctx.enter_context(nc.sbuf_tensor([RPC, 1], f32))
    dps = ctx.enter_context(nc.psum_tensor([RPC, C], f32))
    tps = ctx.enter_context(nc.psum_tensor([C, RPC], f32))

    dma = ctx.enter_context(nc.semaphore())
    s_cmp = ctx.enter_context(nc.semaphore())   # V: chunk m spikes in S       (count m+1)
    s_tr = ctx.enter_context(nc.semaphore())    # T: transpose of chunk m-1    (count m)
    s_spt = ctx.enter_context(nc.semaphore())   # A: spT slice for chunk m-1   (count m)
    s_mm = ctx.enter_context(nc.semaphore())    # T: delta psum for chunk m    (count m)
    s_x = ctx.enter_context(nc.semaphore())     # V: x = y + theta done
    s_p = ctx.enter_context(nc.semaphore())     # A: P sigmoid done

    ge = mybir.AluOpType.is_ge
    mult = mybir.AluOpType.mult
    add = mybir.AluOpType.add

    with nc.Block() as block:

        @block.sync
        def _(sync):
            sync.dma_start(out=y[:, 0:T], in_=y_d[:, :]).then_inc(dma, 16)
            sync.dma_start(out=th[:, :], in_=th_d[:, :]).then_inc(dma, 16)
            sync.dma_start(out=kt[:, :], in_=kt_d[:, :]).then_inc(dma, 16)
            sync.dma_start(out=k4[:, :], in_=k4_d[:, :]).then_inc(dma, 16)
            sync.dma_start(out=ident[:, :], in_=id_d[:, :]).then_inc(dma, 16)
            sync.wait_ge(s_cmp, NCHUNK)
            sync.dma_start(out=out_d[0, :, :], in_=S[:, :]).then_inc(dma, 16)
            sync.wait_ge(s_p, 1)
            sync.dma_start(out=out_d[1, :, :], in_=th[:, :]).then_inc(dma, 16)

        @block.vector
        def _(vector):
            vector.wait_ge(dma, 80)  # ALL input DMAs (completion order varies)
            vector.memset(zb[:, :], 0.0)
            for m in range(NCHUNK):
                t0 = m * C
                if m >= 1:
                    vector.wait_ge(s_mm, m)
                    vector.tensor_add(y[:, t0:t0 + C], y[:, t0:t0 + C], dps[:, :])
                last = None
                for j in range(t0, t0 + C):
                    i = j - t0
                    # Columns t0..t0+i-1 are done; the boundary matmul already
                    # delivered every lag >= max(i+1, 65) to this chunk (and
                    # lags <= 64 spilling into the next chunk come from the
                    # 64-wide floor), so the scatter only needs this window.
                    # The 64 floor also keeps each op longer than the SBUF
                    # write-ack (~67 cyc) so the next cmp never reads stale y.
                    wi = max(C - 1 - i, 64)
                    last = vector.tensor_scalar(S[:, j:j + 1], y[:, j:j + 1], 0.0, None, ge)
                    # drain: the scatter reads S[:, j] as its per-partition
                    # scalar at cycle 0; without a pipe flush that read
                    # races the cmp's SBUF write-ack (~58 cyc).
                    vector.drain()
                    last = vector.scalar_tensor_tensor(
                        y[:, j + 1:j + 1 + wi], kt[:, 1:1 + wi], S[:, j:j + 1],
                        y[:, j + 1:j + 1 + wi], mult, add)
                last.then_inc(s_cmp)  # chunk m spikes complete
            x_i = vector.tensor_add(th[:, :], y[:, 0:T], th[:, :])
            x_i.then_inc(s_x)

        @block.tensor
        def _(tensor):
            tensor.wait_ge(dma, 80)  # k4 + ident loaded
            for m in range(1, NCHUNK):
                tensor.wait_ge(s_cmp, m)       # chunk m-1 spikes ready
                if m >= 2:
                    tensor.wait_ge(s_spt, m - 1)  # tps WAR: copy of m-2 done
                tensor.transpose(tps[:, :], S[:, (m - 1) * C:m * C], ident[:, :]).then_inc(s_tr)
                tensor.wait_ge(s_spt, m)       # spT slice m-1 written
                qs = [q for q in range(4) if m - 4 + q >= 0]
                for qi, q in enumerate(qs):
                    mm = tensor.matmul(
                        dps[:, :],
                        spT[:, (m - 4 + q) * RPC:(m - 3 + q) * RPC],
                        k4[:, q * C:(q + 1) * C],
                        start=(qi == 0), stop=(qi == len(qs) - 1))
                mm.then_inc(s_mm)

        @block.scalar
        def _(scalar):
            for m in range(1, NCHUNK):
                scalar.wait_ge(s_tr, m)
                scalar.activation(spT[:, (m - 1) * RPC:m * RPC], tps[:, :],
                                  mybir.ActivationFunctionType.Copy).then_inc(s_spt)
            scalar.wait_ge(s_x, 1)
            scalar.activation(th[:, :], th[:, :],
                              mybir.ActivationFunctionType.Sigmoid, bias=zb[:, 0:1]).then_inc(s_p)

    ctx.close()
    _CACHE["nc"] = nc
    return nc


def _install_ntff_hook():
    """Shim antenv.axon_hooks (absent in this image) so BASS_TRACE works."""
    import sys, types
    try:
        import antenv.axon_hooks  # noqa: F401
        return
    except ImportError:
        pass
    try:
        if "/root/.axon_site" not in sys.path:
            sys.path.insert(0, "/root/.axon_site")
        from trn_agent_boot.trn_boot import _ntff_profile_via_ctypes
        hook = _ntff_profile_via_ctypes("/opt/axon/libaxon_pjrt.so")
        import antenv
        mod = types.ModuleType("antenv.axon_hooks")
        mod.get_axon_ntff_profile_hook = lambda: hook
        mod.set_axon_ntff_profile_hook = lambda h: None
        antenv.axon_hooks = mod
        sys.modules["antenv.axon_hooks"] = mod
    except Exception:
        pass


# ---------------------------------------------------------------- entry point
def kernel(V, D, w1, b1, w2, b2, W_refract):
    import sys
    if "/opt/trn_rl_repo" not in sys.path:
        sys.path.insert(0, "/opt/trn_rl_repo")
    from concourse.bass_utils import run_bass_kernel_spmd
    global LAST_RESULT
    _install_ntff_hook()

    V = np.asarray(V, np.float32)
    D = np.asarray(D, np.float32)
    theta, _u = _theta()                       # [T, B]
    nn64 = _nn_host(V, D, np.asarray(w1), np.asarray(b1), np.asarray(w2), np.asarray(b2))
    y0 = (nn64 - theta.T.astype(np.float64)).astype(np.float32)   # [B, T]
    th_bt = np.ascontiguousarray(theta.T.astype(np.float32))      # [B, T]

    rk = _refract_kern(np.asarray(W_refract))  # [501]
    wscat = np.zeros(T_NO + 1, np.float32)
    wscat[1:] = rk[::-1]                       # wscat[d] = rk[501-d]
    ktl = np.tile(wscat[None, :], (RPC, 1))    # [8, 502]

    # K4[p, q*C + i] = wscat[512 + i - q*128 - p] for lag in [max(i+1,65), 501]:
    # the boundary matmul covers every lag the in-chunk scatters don't --
    # col i gets lags >= i+1 from older chunks, except lags <= 64 which the
    # previous chunk's 64-wide scatter floor already delivered.
    k4 = np.zeros((C, 4 * C), np.float32)
    p = np.arange(C)[:, None]
    i = np.arange(C)[None, :]
    for q in range(4):
        lag = 512 + i - q * 128 - p
        valid = (lag >= np.maximum(i + 1, 65)) & (lag <= T_NO)
        k4[:, q * C:(q + 1) * C] = np.where(valid, wscat[np.clip(lag, 0, T_NO)], 0.0)
    ident = np.eye(RPC, dtype=np.float32)

    nc = _build_nc()
    in_maps = []
    for c in range(NCORES):
        r = slice(c * RPC, (c + 1) * RPC)
        in_maps.append({
            "y0": np.ascontiguousarray(y0[r]),
            "theta": np.ascontiguousarray(th_bt[r]),
            "ktail": ktl,
            "k4": k4,
            "ident": ident,
        })
    res = run_bass_kernel_spmd(nc, in_maps, core_ids=list(range(NCORES)))
    LAST_RESULT = res
    S = np.concatenate([res.results[c]["out"][0] for c in range(NCORES)], 0)
    P = np.concatenate([res.results[c]["out"][1] for c in range(NCORES)], 0)
    return S, P
